# revision 7
# baseline (speedup 1.0000x reference)
"""Batched TGCN (GCN-GRU) Trainium2 kernel, v2.

Key structure (per core, node-sharded 1250 dst nodes, no cross-core comms):
  - Dense normalized adjacency streamed in bf16; aggregation (A^T X over all
    B*Fin*T=96 feature columns) done as k-chunked matmuls with X stationary.
  - dst nodes split into G=2 groups of 625.  Group g+1's A-stream DMA and
    aggregation matmuls are interleaved into group g's GRU emission so the
    tensor engine stays continuously busy (p-state) and DMA overlaps compute.
  - GRU recurrence in feature-major layout [66, 2500] per group
    (rows 0:64 = H, 64:66 = agg features; columns = batch*625 + node).
    Two 1250-column chunks per step:
      * zr matmul  [66 x 128] -> psum [128, 1250]; sigmoid -> [R|Z] bf16.
      * R*H via fused scalar_tensor_tensor into rh_full rows 0:64.
      * h matmuls for the chunk PAIR write complementary PSUM partition
        halves ([0:64] and [64:128]) so ONE tanh instruction covers both
        chunks ([128, 1250]) -- scalar-engine cost halves vs per-chunk tanh.
      * GRU lerp Hn = ht - Z*(ht - H) as 3 fused stt ops per chunk.
  - Output: Hn (pre-ReLU hidden state) DMA'd per step in bf16.
    Host applies relu + final linear layer + biases (cheap, exact).
"""

import numpy as np
import ml_dtypes

import concourse.mybir as mybir
import concourse.tile as tile
from concourse import bacc
from concourse.bass import ds

F32 = mybir.dt.float32
BF16 = mybir.dt.bfloat16
AF = mybir.ActivationFunctionType
ALU = mybir.AluOpType

# Problem constants (hardcoded per contract)
N_NODES = 10000
FIN = 2
HID = 64
OUT = 16
B = 4
T = 12
N_CORES = 8

P = 128
NPC = N_NODES // N_CORES          # 1250 dst nodes per core
G = 2                             # dst groups per core
NPG = NPC // G                    # 625 dst nodes per group
CELLS = NPG * B                   # 2500 GRU cells per group
CHUNK = CELLS // 2                # 1250 columns per chunk (2 chunks/step)
NFEAT = B * FIN * T               # 96 aggregation features
N_SRC_PAD = ((N_NODES + P - 1) // P) * P   # 10112
KCH = N_SRC_PAD // P              # 79 k-chunks
KB = 8                            # k-chunks per A-stream DMA batch


def build_program():
    nc = bacc.Bacc("TRN2", target_bir_lowering=False, debug=False)

    a_t = nc.dram_tensor("a_t", [P, G, KCH, NPG], BF16, kind="ExternalInput")
    x_feat = nc.dram_tensor("x_feat", [P, KCH, NFEAT], BF16, kind="ExternalInput")
    w_zr = nc.dram_tensor("w_zr", [HID + FIN, 2 * HID], BF16, kind="ExternalInput")
    w_h = nc.dram_tensor("w_h", [HID + FIN, HID], BF16, kind="ExternalInput")
    b_zr = nc.dram_tensor("b_zr", [2 * HID, 1], F32, kind="ExternalInput")
    b_h2 = nc.dram_tensor("b_h2", [2 * HID, 1], F32, kind="ExternalInput")
    out_d = nc.dram_tensor("out", [G, T, HID, CELLS], BF16, kind="ExternalOutput")

    def mm_split(out, lhsT, rhs, start, stop):
        # split along free dim so each matmul stays within a 512-col psum bank
        w = out.shape[-1]
        for o in range(0, w, 512):
            ww = min(512, w - o)
            nc.tensor.matmul(out[:, o : o + ww], lhsT=lhsT, rhs=rhs[:, o : o + ww],
                             start=start, stop=stop)

    with tile.TileContext(nc) as tc:
        with tc.tile_pool(name="persist", bufs=1) as pp:
            x_sb = pp.tile([P, KCH, NFEAT], BF16, tag="x_sb")
            nc.sync.dma_start(x_sb[:], x_feat[:])
            wzr = pp.tile([HID + FIN, 2 * HID], BF16, tag="wzr")
            nc.sync.dma_start(wzr[:], w_zr[:])
            wh = pp.tile([HID + FIN, HID], BF16, tag="wh")
            nc.sync.dma_start(wh[:], w_h[:])
            brz = pp.tile([2 * HID, 1], F32, tag="brz")
            nc.sync.dma_start(brz[:], b_zr[:])
            bh2 = pp.tile([2 * HID, 1], F32, tag="bh2")
            nc.sync.dma_start(bh2[:], b_h2[:])

            # per-group aggregated features, [24 = t*2+fin, 2500 = b*625+n]
            aggT = [pp.tile([FIN * T, CELLS], BF16, tag=f"aggT{g}", name=f"aggT{g}")
                    for g in range(G)]
            # per-group GRU state (rows 0:64 H, 64:66 agg), ping-pong over t
            h_bufs = [[pp.tile([HID + FIN, CELLS], BF16, tag=f"h{g}_{i}",
                               name=f"h{g}_{i}") for i in range(2)] for g in range(G)]
            rh_bufs = [[pp.tile([HID + FIN, CELLS], BF16, tag=f"rh{g}_{i}",
                                name=f"rh{g}_{i}") for i in range(2)] for g in range(G)]
            for g in range(G):
                nc.gpsimd.memset(h_bufs[g][0][:HID, :], 0.0)

            with (
                tc.tile_pool(name="astream", bufs=3) as ap_,
                tc.tile_pool(name="apsum", bufs=1, space="PSUM") as aps,
                tc.tile_pool(name="gpsum", bufs=2, space="PSUM") as gps,
                tc.tile_pool(name="work", bufs=6) as wp,
                tc.tile_pool(name="htp", bufs=2) as hp_,
            ):
                # ---- stage-1 emission helpers (aggregation for group g) ----
                agg_state = {}

                def agg_begin(g):
                    agg_state[g] = aps.tile([NFEAT, NPG], F32, tag="agp",
                                            name=f"agp{g}")

                def agg_k(g, k):
                    # one k-chunk of the aggregation for group g
                    if k % KB == 0:
                        kb = min(KB, KCH - k)
                        a_sb = ap_.tile([P, KB, NPG], BF16, tag="a_sb")
                        agg_state[(g, "a")] = a_sb
                        nc.sync.dma_start(a_sb[:, :kb], a_t[:, g, k : k + kb, :])
                    a_sb = agg_state[(g, "a")]
                    mm_split(agg_state[g], x_sb[:, k, :], a_sb[:, k % KB, :],
                             start=(k == 0), stop=(k == KCH - 1))

                def agg_finish(g):
                    # evict psum (aligned copy), then de-interleave batch via
                    # SBUF->SBUF DMA: aggT[g][t2f, b*625+n] = stage[b*24+t2f, n]
                    psum = agg_state.pop(g)
                    stage = wp.tile([NFEAT, NPG], BF16, tag="astg", name=f"astg{g}")
                    nc.vector.tensor_copy(stage, psum)
                    for b in range(B):
                        nc.sync.dma_start(
                            aggT[g][:, ds(b * NPG, NPG)],
                            stage[b * (FIN * T) : (b + 1) * (FIN * T), :])

                # ---- stage-2 (GRU) emission for one (g, t) ----
                def gru_step(g, t, sprinkle):
                    h_prev, h_next = h_bufs[g][t % 2], h_bufs[g][(t + 1) % 2]
                    rh_full = rh_bufs[g][t % 2]
                    # agg feature rows for this step
                    nc.sync.dma_start(h_prev[HID:, :], aggT[g][2 * t : 2 * t + 2, :])
                    nc.sync.dma_start(rh_full[HID:, :], aggT[g][2 * t : 2 * t + 2, :])

                    sigs = []
                    for c in range(2):
                        cc = ds(c * CHUNK, CHUNK)
                        pz = gps.tile([2 * HID, CHUNK], F32, tag="gp", name=f"pz{g}_{t}_{c}")
                        mm_split(pz, wzr[:], h_prev[:, cc], True, True)
                        sig = wp.tile([2 * HID, CHUNK], BF16, tag="sig", name=f"sig{g}_{t}_{c}")
                        nc.scalar.activation(sig, pz, AF.Sigmoid, bias=brz[:, 0:1])
                        # rh = R * H
                        nc.vector.tensor_tensor(
                            rh_full[:HID, cc], in0=sig[:HID, :],
                            in1=h_prev[:HID, cc], op=ALU.mult)
                        sigs.append(sig)

                    for f in sprinkle[:4]:
                        f()

                    # h matmul + tanh per chunk (all tiles partition-base 0)
                    hts = []
                    for c in range(2):
                        cc = ds(c * CHUNK, CHUNK)
                        ph = gps.tile([2 * HID, CHUNK], F32, tag="gp", name=f"ph{g}_{t}_{c}")
                        mm_split(ph[0:HID, :], wh[:], rh_full[:, cc], True, True)
                        ht = hp_.tile([HID, CHUNK], BF16, tag="ht", name=f"ht{g}_{t}_{c}")
                        nc.scalar.activation(ht, ph[0:HID, :], AF.Tanh,
                                             bias=bh2[:HID, 0:1])
                        hts.append(ht)

                    for f in sprinkle[4:]:
                        f()

                    # lerp per chunk: Hn = ht - Z*(ht - H).  The Z-half of sig
                    # lives at partition base 64; the verifier requires matched
                    # input bases, so d0 is written at base 64 (baseline trick)
                    # and d1 at base 0.  One sub per step runs on GpSimd.
                    for c in range(2):
                        cc = ds(c * CHUNK, CHUNK)
                        ht = hts[c]
                        d0f = wp.tile([2 * HID, CHUNK], BF16, tag="d0", name=f"d0_{g}_{t}_{c}")
                        eng = nc.gpsimd if c == 0 else nc.vector
                        eng.tensor_tensor(
                            d0f[HID:, :], in0=ht, in1=h_prev[:HID, cc], op=ALU.subtract)
                        d1 = wp.tile([HID, CHUNK], BF16, tag="d1", name=f"d1_{g}_{t}_{c}")
                        nc.vector.tensor_tensor(
                            d1, in0=sigs[c][HID:, :], in1=d0f[HID:, :], op=ALU.mult)
                        nc.vector.tensor_tensor(
                            h_next[:HID, cc], in0=ht, in1=d1, op=ALU.subtract)

                    # emit hidden state (host applies relu + linear head)
                    nc.sync.dma_start(out_d[g, t], h_next[:HID, :])

                # ---------------- schedule ----------------
                # stage 1 for group 0 (DMA-paced; PE interleaves as data lands)
                agg_begin(0)
                for k in range(KCH):
                    agg_k(0, k)
                agg_finish(0)

                # GRU(0) with stage-1(1) sprinkled in; then GRU(1)
                agg_begin(1)
                nk = [0]

                def mk_sprinkle(g, n):
                    fs = []
                    for _ in range(n):
                        if nk[0] < KCH:
                            k = nk[0]
                            nk[0] += 1
                            fs.append(lambda k=k: agg_k(g, k))
                    return fs

                for t in range(T):
                    gru_step(0, t, mk_sprinkle(1, 7))
                while nk[0] < KCH:
                    agg_k(1, nk[0])
                    nk[0] += 1
                agg_finish(1)
                for t in range(T):
                    gru_step(1, t, [])

    nc.compile()
    return nc


def _prep_host(x, edge_index, edge_weight, Wz, bz, Wr, br, Wh, bh,
               Lz_w, Lz_b, Lr_w, Lr_b, Lh_w, Lh_b, lin_w, lin_b):
    """Host-side preprocessing: norm, dense A, feature reorder, weight folding."""
    bf16 = ml_dtypes.bfloat16
    f32 = np.float32

    src = np.asarray(edge_index[0], dtype=np.int64)
    dst = np.asarray(edge_index[1], dtype=np.int64)
    w = np.asarray(edge_weight, dtype=f32)
    loop = np.arange(N_NODES, dtype=np.int64)
    src_a = np.concatenate([src, loop])
    dst_a = np.concatenate([dst, loop])
    w_a = np.concatenate([w, np.ones(N_NODES, f32)])
    deg = np.zeros(N_NODES, f32)
    np.add.at(deg, dst_a, w_a)
    dinv = np.where(deg > 0, 1.0 / np.sqrt(deg), 0.0).astype(f32)
    norm = dinv[src_a] * w_a * dinv[dst_a]

    A = np.zeros((N_SRC_PAD, N_NODES), f32)   # A[src, dst]
    np.add.at(A, (src_a, dst_a), norm)
    A_bf = A.astype(bf16)

    # X rows: [n_src_pad, 96], f = b*24 + t*2 + fin
    X = np.zeros((N_SRC_PAD, B * T * FIN), f32)
    X[:N_NODES] = np.transpose(np.asarray(x, f32), (1, 0, 3, 2)).reshape(N_NODES, -1)
    x_feat = np.ascontiguousarray(
        X.astype(bf16).reshape(KCH, P, B * T * FIN).transpose(1, 0, 2))

    Wz, Wr, Wh = np.asarray(Wz, f32), np.asarray(Wr, f32), np.asarray(Wh, f32)
    Lz_w, Lr_w, Lh_w = np.asarray(Lz_w, f32), np.asarray(Lr_w, f32), np.asarray(Lh_w, f32)
    Az, Ar, Ah = Wz @ Lz_w[:HID], Wr @ Lr_w[:HID], Wh @ Lh_w[:HID]
    Lz2, Lr2, Lh2 = Lz_w[HID:], Lr_w[HID:], Lh_w[HID:]
    bz_f = np.asarray(bz, f32) @ Lz_w[:HID] + np.asarray(Lz_b, f32)
    br_f = np.asarray(br, f32) @ Lr_w[:HID] + np.asarray(Lr_b, f32)
    bh_f = np.asarray(bh, f32) @ Lh_w[:HID] + np.asarray(Lh_b, f32)

    w_zr_np = np.concatenate([np.concatenate([Lr2, Lz2], axis=1),
                              np.concatenate([Ar, Az], axis=1)], axis=0)
    w_h_np = np.concatenate([Lh2, Ah], axis=0)
    common = {
        "x_feat": x_feat,
        "w_zr": np.ascontiguousarray(w_zr_np).astype(bf16),
        "w_h": np.ascontiguousarray(w_h_np).astype(bf16),
        "b_zr": np.concatenate([br_f, bz_f]).astype(f32).reshape(-1, 1),
        "b_h2": np.concatenate([bh_f, bh_f]).astype(f32).reshape(-1, 1),
    }
    in_maps = []
    for c in range(N_CORES):
        blk = A_bf[:, c * NPC : (c + 1) * NPC]            # [10112, 1250]
        # [P, G, KCH, NPG]
        a_core = np.ascontiguousarray(
            blk.reshape(KCH, P, G, NPG).transpose(1, 2, 0, 3))
        in_maps.append(dict(common, a_t=a_core))
    return in_maps


_CACHED_NC = None


def kernel(**inputs) -> np.ndarray:
    global _CACHED_NC
    from concourse.bass_utils import run_bass_kernel_spmd

    in_maps = _prep_host(**inputs)
    if _CACHED_NC is None:
        _CACHED_NC = build_program()
    res = run_bass_kernel_spmd(_CACHED_NC, in_maps, core_ids=list(range(N_CORES)))

    lin_w = np.asarray(inputs["lin_w"], np.float32)
    lin_b = np.asarray(inputs["lin_b"], np.float32)
    full = np.empty((B, T, N_NODES, OUT), np.float32)
    for c, r in enumerate(res.results):
        hn = np.maximum(r["out"].astype(np.float32), 0.0)   # [G, T, 64, 2500]
        hn = hn.reshape(G, T, HID, B, NPG).transpose(3, 1, 0, 4, 2)  # b,t,g,n,h
        o = hn.reshape(-1, HID) @ lin_w + lin_b
        full[:, :, c * NPC : (c + 1) * NPC, :] = o.reshape(B, T, NPC, OUT)
    return full


# revision 9
# speedup vs baseline: 1.1688x; 1.1688x over previous
"""Batched TGCN (GCN-GRU) Trainium2 kernel, v2.

Key structure (per core, node-sharded 1250 dst nodes, no cross-core comms):
  - Dense normalized adjacency streamed in bf16; aggregation (A^T X over all
    B*Fin*T=96 feature columns) done as k-chunked matmuls with X stationary.
  - dst nodes split into G=2 groups of 625.  Group g+1's A-stream DMA and
    aggregation matmuls are interleaved into group g's GRU emission so the
    tensor engine stays continuously busy (p-state) and DMA overlaps compute.
  - GRU recurrence in feature-major layout [66, 2500] per group
    (rows 0:64 = H, 64:66 = agg features; columns = batch*625 + node).
    Two 1250-column chunks per step:
      * zr matmul  [66 x 128] -> psum [128, 1250]; sigmoid -> [R|Z] bf16.
      * R*H via fused scalar_tensor_tensor into rh_full rows 0:64.
      * h matmuls for the chunk PAIR write complementary PSUM partition
        halves ([0:64] and [64:128]) so ONE tanh instruction covers both
        chunks ([128, 1250]) -- scalar-engine cost halves vs per-chunk tanh.
      * GRU lerp Hn = ht - Z*(ht - H) as 3 fused stt ops per chunk.
  - Output: Hn (pre-ReLU hidden state) DMA'd per step in bf16.
    Host applies relu + final linear layer + biases (cheap, exact).
"""

import numpy as np
import ml_dtypes

import concourse.mybir as mybir
import concourse.tile as tile
from concourse import bacc
from concourse.bass import ds

F32 = mybir.dt.float32
BF16 = mybir.dt.bfloat16
AF = mybir.ActivationFunctionType
ALU = mybir.AluOpType

# Problem constants (hardcoded per contract)
N_NODES = 10000
FIN = 2
HID = 64
OUT = 16
B = 4
T = 12
N_CORES = 8

P = 128
NPC = N_NODES // N_CORES          # 1250 dst nodes per core
G = 2                             # dst groups per core
NPG = NPC // G                    # 625 dst nodes per group
CELLS = NPG * B                   # 2500 GRU cells per group
CHUNK = CELLS // 2                # 1250 columns per chunk (2 chunks/step)
NFEAT = B * FIN * T               # 96 aggregation features
N_SRC_PAD = ((N_NODES + P - 1) // P) * P   # 10112
KCH = N_SRC_PAD // P              # 79 k-chunks
KB = 8                            # k-chunks per A-stream DMA batch


def build_program():
    nc = bacc.Bacc("TRN2", target_bir_lowering=False, debug=False)

    a_t = nc.dram_tensor("a_t", [P, G, KCH, NPG], BF16, kind="ExternalInput")
    x_feat = nc.dram_tensor("x_feat", [P, KCH, NFEAT], BF16, kind="ExternalInput")
    w_zr = nc.dram_tensor("w_zr", [HID + FIN, 2 * HID], BF16, kind="ExternalInput")
    w_h = nc.dram_tensor("w_h", [HID + FIN, HID], BF16, kind="ExternalInput")
    b_zr = nc.dram_tensor("b_zr", [2 * HID, 1], F32, kind="ExternalInput")
    b_h2 = nc.dram_tensor("b_h2", [2 * HID, 1], F32, kind="ExternalInput")
    out_d = nc.dram_tensor("out", [G, T, HID, CELLS], BF16, kind="ExternalOutput")

    def mm_split(out, lhsT, rhs, start, stop):
        # split along free dim so each matmul stays within a 512-col psum bank
        w = out.shape[-1]
        for o in range(0, w, 512):
            ww = min(512, w - o)
            nc.tensor.matmul(out[:, o : o + ww], lhsT=lhsT, rhs=rhs[:, o : o + ww],
                             start=start, stop=stop)

    with tile.TileContext(nc) as tc:
        with tc.tile_pool(name="persist", bufs=1) as pp:
            x_sb = pp.tile([P, KCH, NFEAT], BF16, tag="x_sb")
            nc.sync.dma_start(x_sb[:], x_feat[:])
            wzr = pp.tile([HID + FIN, 2 * HID], BF16, tag="wzr")
            nc.sync.dma_start(wzr[:], w_zr[:])
            wh = pp.tile([HID + FIN, HID], BF16, tag="wh")
            nc.sync.dma_start(wh[:], w_h[:])
            brz = pp.tile([2 * HID, 1], F32, tag="brz")
            nc.sync.dma_start(brz[:], b_zr[:])
            bh2 = pp.tile([2 * HID, 1], F32, tag="bh2")
            nc.sync.dma_start(bh2[:], b_h2[:])

            # per-group aggregated features, [24 = t*2+fin, 2500 = b*625+n]
            aggT = [pp.tile([FIN * T, CELLS], BF16, tag=f"aggT{g}", name=f"aggT{g}")
                    for g in range(G)]
            # per-group GRU state (rows 0:64 H, 64:66 agg), ping-pong over t
            h_bufs = [[pp.tile([HID + FIN, CELLS], BF16, tag=f"h{g}_{i}",
                               name=f"h{g}_{i}") for i in range(2)] for g in range(G)]
            rh_bufs = [[pp.tile([HID + FIN, CELLS], BF16, tag=f"rh{g}_{i}",
                                name=f"rh{g}_{i}") for i in range(2)] for g in range(G)]
            for g in range(G):
                nc.gpsimd.memset(h_bufs[g][0][:HID, :], 0.0)

            with (
                tc.tile_pool(name="astream", bufs=3) as ap_,
                tc.tile_pool(name="apsum", bufs=1, space="PSUM") as aps,
                tc.tile_pool(name="gpsum", bufs=2, space="PSUM") as gps,
                tc.tile_pool(name="work", bufs=6) as wp,
                tc.tile_pool(name="htp", bufs=2) as hp_,
            ):
                # ---- stage-1 emission helpers (aggregation for group g) ----
                agg_state = {}

                def agg_begin(g):
                    agg_state[g] = aps.tile([NFEAT, NPG], F32, tag="agp",
                                            name=f"agp{g}")

                def agg_k(g, k):
                    # one k-chunk of the aggregation for group g
                    if k % KB == 0:
                        kb = min(KB, KCH - k)
                        a_sb = ap_.tile([P, KB, NPG], BF16, tag="a_sb")
                        agg_state[(g, "a")] = a_sb
                        nc.sync.dma_start(a_sb[:, :kb], a_t[:, g, k : k + kb, :])
                    a_sb = agg_state[(g, "a")]
                    mm_split(agg_state[g], x_sb[:, k, :], a_sb[:, k % KB, :],
                             start=(k == 0), stop=(k == KCH - 1))

                def agg_finish(g):
                    # evict psum (aligned copy), then de-interleave batch via
                    # SBUF->SBUF DMA: aggT[g][t2f, b*625+n] = stage[b*24+t2f, n]
                    psum = agg_state.pop(g)
                    stage = wp.tile([NFEAT, NPG], BF16, tag="astg", name=f"astg{g}")
                    nc.vector.tensor_copy(stage, psum)
                    for b in range(B):
                        nc.sync.dma_start(
                            aggT[g][:, ds(b * NPG, NPG)],
                            stage[b * (FIN * T) : (b + 1) * (FIN * T), :])

                # ---- stage-2 (GRU) emission for one (g, t) ----
                def gru_step(g, t, sprinkle):
                    h_prev, h_next = h_bufs[g][t % 2], h_bufs[g][(t + 1) % 2]
                    rh_full = rh_bufs[g][t % 2]
                    # agg feature rows for this step
                    nc.sync.dma_start(h_prev[HID:, :], aggT[g][2 * t : 2 * t + 2, :])
                    nc.sync.dma_start(rh_full[HID:, :], aggT[g][2 * t : 2 * t + 2, :])

                    sigs = []
                    for c in range(2):
                        cc = ds(c * CHUNK, CHUNK)
                        pz = gps.tile([2 * HID, CHUNK], F32, tag="gp", name=f"pz{g}_{t}_{c}")
                        mm_split(pz, wzr[:], h_prev[:, cc], True, True)
                        sig = wp.tile([2 * HID, CHUNK], BF16, tag="sig", name=f"sig{g}_{t}_{c}")
                        nc.scalar.activation(sig, pz, AF.Sigmoid, bias=brz[:, 0:1])
                        # rh = R * H
                        nc.vector.tensor_tensor(
                            rh_full[:HID, cc], in0=sig[:HID, :],
                            in1=h_prev[:HID, cc], op=ALU.mult)
                        sigs.append(sig)

                    for f in sprinkle[:4]:
                        f()

                    # h matmul + tanh per chunk (all tiles partition-base 0)
                    hts = []
                    for c in range(2):
                        cc = ds(c * CHUNK, CHUNK)
                        ph = gps.tile([2 * HID, CHUNK], F32, tag="gp", name=f"ph{g}_{t}_{c}")
                        mm_split(ph[0:HID, :], wh[:], rh_full[:, cc], True, True)
                        ht = hp_.tile([HID, CHUNK], BF16, tag="ht", name=f"ht{g}_{t}_{c}")
                        nc.scalar.activation(ht, ph[0:HID, :], AF.Tanh,
                                             bias=bh2[:HID, 0:1])
                        hts.append(ht)

                    for f in sprinkle[4:]:
                        f()

                    # lerp per chunk: Hn = ht - Z*(ht - H).  The Z-half of sig
                    # lives at partition base 64; the verifier requires matched
                    # input bases, so d0 is written at base 64 (baseline trick)
                    # and d1 at base 0.  One sub per step runs on GpSimd.
                    for c in range(2):
                        cc = ds(c * CHUNK, CHUNK)
                        ht = hts[c]
                        d0f = wp.tile([2 * HID, CHUNK], BF16, tag="d0", name=f"d0_{g}_{t}_{c}")
                        nc.vector.tensor_tensor(
                            d0f[HID:, :], in0=ht, in1=h_prev[:HID, cc], op=ALU.subtract)
                        d1 = wp.tile([HID, CHUNK], BF16, tag="d1", name=f"d1_{g}_{t}_{c}")
                        nc.vector.tensor_tensor(
                            d1, in0=sigs[c][HID:, :], in1=d0f[HID:, :], op=ALU.mult)
                        nc.vector.tensor_tensor(
                            h_next[:HID, cc], in0=ht, in1=d1, op=ALU.subtract)

                    # emit hidden state (host applies relu + linear head)
                    nc.sync.dma_start(out_d[g, t], h_next[:HID, :])

                # ---------------- schedule ----------------
                # stage 1 for group 0 (DMA-paced; PE interleaves as data lands)
                agg_begin(0)
                for k in range(KCH):
                    agg_k(0, k)
                agg_finish(0)

                # GRU(0) with stage-1(1) sprinkled in; then GRU(1)
                agg_begin(1)
                nk = [0]

                def mk_sprinkle(g, n):
                    fs = []
                    for _ in range(n):
                        if nk[0] < KCH:
                            k = nk[0]
                            nk[0] += 1
                            fs.append(lambda k=k: agg_k(g, k))
                    return fs

                # group-0 GRU steps 0..6 absorb all of group-1's aggregation;
                # then the two groups' recurrences interleave so their
                # independent per-chunk dependency chains hide each other's
                # latency (the recurrence is latency-bound, not engine-bound).
                for t in range(7):
                    gru_step(0, t, mk_sprinkle(1, 12))
                while nk[0] < KCH:
                    agg_k(1, nk[0])
                    nk[0] += 1
                agg_finish(1)
                for i, (g, t) in enumerate(
                        [(0, 7), (1, 0), (0, 8), (1, 1), (0, 9), (1, 2),
                         (0, 10), (1, 3), (0, 11), (1, 4)]
                        + [(1, t) for t in range(5, T)]):
                    gru_step(g, t, [])

    nc.compile()
    return nc


def _prep_host(x, edge_index, edge_weight, Wz, bz, Wr, br, Wh, bh,
               Lz_w, Lz_b, Lr_w, Lr_b, Lh_w, Lh_b, lin_w, lin_b):
    """Host-side preprocessing: norm, dense A, feature reorder, weight folding."""
    bf16 = ml_dtypes.bfloat16
    f32 = np.float32

    src = np.asarray(edge_index[0], dtype=np.int64)
    dst = np.asarray(edge_index[1], dtype=np.int64)
    w = np.asarray(edge_weight, dtype=f32)
    loop = np.arange(N_NODES, dtype=np.int64)
    src_a = np.concatenate([src, loop])
    dst_a = np.concatenate([dst, loop])
    w_a = np.concatenate([w, np.ones(N_NODES, f32)])
    deg = np.zeros(N_NODES, f32)
    np.add.at(deg, dst_a, w_a)
    dinv = np.where(deg > 0, 1.0 / np.sqrt(deg), 0.0).astype(f32)
    norm = dinv[src_a] * w_a * dinv[dst_a]

    A = np.zeros((N_SRC_PAD, N_NODES), f32)   # A[src, dst]
    np.add.at(A, (src_a, dst_a), norm)
    A_bf = A.astype(bf16)

    # X rows: [n_src_pad, 96], f = b*24 + t*2 + fin
    X = np.zeros((N_SRC_PAD, B * T * FIN), f32)
    X[:N_NODES] = np.transpose(np.asarray(x, f32), (1, 0, 3, 2)).reshape(N_NODES, -1)
    x_feat = np.ascontiguousarray(
        X.astype(bf16).reshape(KCH, P, B * T * FIN).transpose(1, 0, 2))

    Wz, Wr, Wh = np.asarray(Wz, f32), np.asarray(Wr, f32), np.asarray(Wh, f32)
    Lz_w, Lr_w, Lh_w = np.asarray(Lz_w, f32), np.asarray(Lr_w, f32), np.asarray(Lh_w, f32)
    Az, Ar, Ah = Wz @ Lz_w[:HID], Wr @ Lr_w[:HID], Wh @ Lh_w[:HID]
    Lz2, Lr2, Lh2 = Lz_w[HID:], Lr_w[HID:], Lh_w[HID:]
    bz_f = np.asarray(bz, f32) @ Lz_w[:HID] + np.asarray(Lz_b, f32)
    br_f = np.asarray(br, f32) @ Lr_w[:HID] + np.asarray(Lr_b, f32)
    bh_f = np.asarray(bh, f32) @ Lh_w[:HID] + np.asarray(Lh_b, f32)

    w_zr_np = np.concatenate([np.concatenate([Lr2, Lz2], axis=1),
                              np.concatenate([Ar, Az], axis=1)], axis=0)
    w_h_np = np.concatenate([Lh2, Ah], axis=0)
    common = {
        "x_feat": x_feat,
        "w_zr": np.ascontiguousarray(w_zr_np).astype(bf16),
        "w_h": np.ascontiguousarray(w_h_np).astype(bf16),
        "b_zr": np.concatenate([br_f, bz_f]).astype(f32).reshape(-1, 1),
        "b_h2": np.concatenate([bh_f, bh_f]).astype(f32).reshape(-1, 1),
    }
    in_maps = []
    for c in range(N_CORES):
        blk = A_bf[:, c * NPC : (c + 1) * NPC]            # [10112, 1250]
        # [P, G, KCH, NPG]
        a_core = np.ascontiguousarray(
            blk.reshape(KCH, P, G, NPG).transpose(1, 2, 0, 3))
        in_maps.append(dict(common, a_t=a_core))
    return in_maps


_CACHED_NC = None


def kernel(**inputs) -> np.ndarray:
    global _CACHED_NC
    from concourse.bass_utils import run_bass_kernel_spmd

    in_maps = _prep_host(**inputs)
    if _CACHED_NC is None:
        _CACHED_NC = build_program()
    res = run_bass_kernel_spmd(_CACHED_NC, in_maps, core_ids=list(range(N_CORES)))

    lin_w = np.asarray(inputs["lin_w"], np.float32)
    lin_b = np.asarray(inputs["lin_b"], np.float32)
    full = np.empty((B, T, N_NODES, OUT), np.float32)
    for c, r in enumerate(res.results):
        hn = np.maximum(r["out"].astype(np.float32), 0.0)   # [G, T, 64, 2500]
        hn = hn.reshape(G, T, HID, B, NPG).transpose(3, 1, 0, 4, 2)  # b,t,g,n,h
        o = hn.reshape(-1, HID) @ lin_w + lin_b
        full[:, :, c * NPC : (c + 1) * NPC, :] = o.reshape(B, T, NPC, OUT)
    return full


# revision 10
# speedup vs baseline: 1.2961x; 1.1090x over previous
"""Batched TGCN (GCN-GRU) Trainium2 kernel, v2.

Key structure (per core, node-sharded 1250 dst nodes, no cross-core comms):
  - Dense normalized adjacency streamed in bf16; aggregation (A^T X over all
    B*Fin*T=96 feature columns) done as k-chunked matmuls with X stationary.
  - dst nodes split into G=2 groups of 625.  Group g+1's A-stream DMA and
    aggregation matmuls are interleaved into group g's GRU emission so the
    tensor engine stays continuously busy (p-state) and DMA overlaps compute.
  - GRU recurrence in feature-major layout [66, 2500] per group
    (rows 0:64 = H, 64:66 = agg features; columns = batch*625 + node).
    Two 1250-column chunks per step:
      * zr matmul  [66 x 128] -> psum [128, 1250]; sigmoid -> [R|Z] bf16.
      * R*H via fused scalar_tensor_tensor into rh_full rows 0:64.
      * h matmuls for the chunk PAIR write complementary PSUM partition
        halves ([0:64] and [64:128]) so ONE tanh instruction covers both
        chunks ([128, 1250]) -- scalar-engine cost halves vs per-chunk tanh.
      * GRU lerp Hn = ht - Z*(ht - H) as 3 fused stt ops per chunk.
  - Output: Hn (pre-ReLU hidden state) DMA'd per step in bf16.
    Host applies relu + final linear layer + biases (cheap, exact).
"""

import numpy as np
import ml_dtypes

import concourse.mybir as mybir
import concourse.tile as tile
from concourse import bacc
from concourse.bass import ds

F32 = mybir.dt.float32
BF16 = mybir.dt.bfloat16
AF = mybir.ActivationFunctionType
ALU = mybir.AluOpType

# Problem constants (hardcoded per contract)
N_NODES = 10000
FIN = 2
HID = 64
OUT = 16
B = 4
T = 12
N_CORES = 8

P = 128
NPC = N_NODES // N_CORES          # 1250 dst nodes per core
G = 3                             # dst groups per core
NPG = 418                         # dst nodes per group (3*418 = 1254, padded)
CELLS = NPG * B                   # 1672 GRU cells per group
CHUNK = CELLS // 2                # 836 columns per chunk (2 chunks/step)
NFEAT = B * FIN * T               # 96 aggregation features
N_SRC_PAD = ((N_NODES + P - 1) // P) * P   # 10112
KCH = N_SRC_PAD // P              # 79 k-chunks
KB = 8                            # k-chunks per A-stream DMA batch


def build_program():
    nc = bacc.Bacc("TRN2", target_bir_lowering=False, debug=False)

    a_t = nc.dram_tensor("a_t", [P, G, KCH, NPG], BF16, kind="ExternalInput")
    x_feat = nc.dram_tensor("x_feat", [P, KCH, NFEAT], BF16, kind="ExternalInput")
    w_zr = nc.dram_tensor("w_zr", [HID + FIN, 2 * HID], BF16, kind="ExternalInput")
    w_h = nc.dram_tensor("w_h", [HID + FIN, HID], BF16, kind="ExternalInput")
    b_zr = nc.dram_tensor("b_zr", [2 * HID, 1], F32, kind="ExternalInput")
    b_h2 = nc.dram_tensor("b_h2", [2 * HID, 1], F32, kind="ExternalInput")
    out_d = nc.dram_tensor("out", [G, T, HID, CELLS], BF16, kind="ExternalOutput")

    def mm_split(out, lhsT, rhs, start, stop):
        # split along free dim so each matmul stays within a 512-col psum bank
        w = out.shape[-1]
        for o in range(0, w, 512):
            ww = min(512, w - o)
            nc.tensor.matmul(out[:, o : o + ww], lhsT=lhsT, rhs=rhs[:, o : o + ww],
                             start=start, stop=stop)

    with tile.TileContext(nc) as tc:
        with tc.tile_pool(name="persist", bufs=1) as pp:
            x_sb = pp.tile([P, KCH, NFEAT], BF16, tag="x_sb")
            nc.sync.dma_start(x_sb[:], x_feat[:])
            wzr = pp.tile([HID + FIN, 2 * HID], BF16, tag="wzr")
            nc.sync.dma_start(wzr[:], w_zr[:])
            wh = pp.tile([HID + FIN, HID], BF16, tag="wh")
            nc.sync.dma_start(wh[:], w_h[:])
            brz = pp.tile([2 * HID, 1], F32, tag="brz")
            nc.sync.dma_start(brz[:], b_zr[:])
            bh2 = pp.tile([2 * HID, 1], F32, tag="bh2")
            nc.sync.dma_start(bh2[:], b_h2[:])

            # per-group aggregated features, [24 = t*2+fin, 2500 = b*625+n]
            aggT = [pp.tile([FIN * T, CELLS], BF16, tag=f"aggT{g}", name=f"aggT{g}")
                    for g in range(G)]
            # per-group GRU state (rows 0:64 H, 64:66 agg), ping-pong over t
            h_bufs = [[pp.tile([HID + FIN, CELLS], BF16, tag=f"h{g}_{i}",
                               name=f"h{g}_{i}") for i in range(2)] for g in range(G)]
            rh_bufs = [[pp.tile([HID + FIN, CELLS], BF16, tag=f"rh{g}_{i}",
                                name=f"rh{g}_{i}") for i in range(2)] for g in range(G)]
            for g in range(G):
                nc.gpsimd.memset(h_bufs[g][0][:HID, :], 0.0)

            with (
                tc.tile_pool(name="astream", bufs=3) as ap_,
                tc.tile_pool(name="apsum", bufs=1, space="PSUM") as aps,
                tc.tile_pool(name="gpsum", bufs=3, space="PSUM") as gps,
                tc.tile_pool(name="work", bufs=6) as wp,
                tc.tile_pool(name="htp", bufs=2) as hp_,
            ):
                # ---- stage-1 emission helpers (aggregation for group g) ----
                agg_state = {}

                def agg_begin(g):
                    agg_state[g] = aps.tile([NFEAT, NPG], F32, tag="agp",
                                            name=f"agp{g}")

                def agg_k(g, k):
                    # one k-chunk of the aggregation for group g
                    if k % KB == 0:
                        kb = min(KB, KCH - k)
                        a_sb = ap_.tile([P, KB, NPG], BF16, tag="a_sb")
                        agg_state[(g, "a")] = a_sb
                        nc.sync.dma_start(a_sb[:, :kb], a_t[:, g, k : k + kb, :])
                    a_sb = agg_state[(g, "a")]
                    mm_split(agg_state[g], x_sb[:, k, :], a_sb[:, k % KB, :],
                             start=(k == 0), stop=(k == KCH - 1))

                def agg_finish(g):
                    # evict psum (aligned copy), then de-interleave batch via
                    # SBUF->SBUF DMA: aggT[g][t2f, b*625+n] = stage[b*24+t2f, n]
                    psum = agg_state.pop(g)
                    stage = wp.tile([NFEAT, NPG], BF16, tag="astg", name=f"astg{g}")
                    nc.vector.tensor_copy(stage, psum)
                    for b in range(B):
                        nc.sync.dma_start(
                            aggT[g][:, ds(b * NPG, NPG)],
                            stage[b * (FIN * T) : (b + 1) * (FIN * T), :])

                # ---- stage-2 (GRU) emission for one (g, t) ----
                def gru_step(g, t, sprinkle):
                    h_prev, h_next = h_bufs[g][t % 2], h_bufs[g][(t + 1) % 2]
                    rh_full = rh_bufs[g][t % 2]
                    # agg feature rows for this step
                    nc.sync.dma_start(h_prev[HID:, :], aggT[g][2 * t : 2 * t + 2, :])
                    nc.sync.dma_start(rh_full[HID:, :], aggT[g][2 * t : 2 * t + 2, :])

                    sigs = []
                    for c in range(2):
                        cc = ds(c * CHUNK, CHUNK)
                        pz = gps.tile([2 * HID, CHUNK], F32, tag="gp", name=f"pz{g}_{t}_{c}")
                        mm_split(pz, wzr[:], h_prev[:, cc], True, True)
                        sig = wp.tile([2 * HID, CHUNK], BF16, tag="sig", name=f"sig{g}_{t}_{c}")
                        nc.scalar.activation(sig, pz, AF.Sigmoid, bias=brz[:, 0:1])
                        # rh = R * H
                        nc.vector.tensor_tensor(
                            rh_full[:HID, cc], in0=sig[:HID, :],
                            in1=h_prev[:HID, cc], op=ALU.mult)
                        sigs.append(sig)

                    for f in sprinkle[:4]:
                        f()

                    # h matmul + tanh per chunk (all tiles partition-base 0)
                    hts = []
                    for c in range(2):
                        cc = ds(c * CHUNK, CHUNK)
                        ph = gps.tile([2 * HID, CHUNK], F32, tag="gp", name=f"ph{g}_{t}_{c}")
                        mm_split(ph[0:HID, :], wh[:], rh_full[:, cc], True, True)
                        ht = hp_.tile([HID, CHUNK], BF16, tag="ht", name=f"ht{g}_{t}_{c}")
                        nc.scalar.activation(ht, ph[0:HID, :], AF.Tanh,
                                             bias=bh2[:HID, 0:1])
                        hts.append(ht)

                    for f in sprinkle[4:]:
                        f()

                    # lerp per chunk: Hn = ht - Z*(ht - H).  The Z-half of sig
                    # lives at partition base 64; the verifier requires matched
                    # input bases, so d0 is written at base 64 (baseline trick)
                    # and d1 at base 0.  One sub per step runs on GpSimd.
                    for c in range(2):
                        cc = ds(c * CHUNK, CHUNK)
                        ht = hts[c]
                        d0f = wp.tile([2 * HID, CHUNK], BF16, tag="d0", name=f"d0_{g}_{t}_{c}")
                        nc.vector.tensor_tensor(
                            d0f[HID:, :], in0=ht, in1=h_prev[:HID, cc], op=ALU.subtract)
                        d1 = wp.tile([HID, CHUNK], BF16, tag="d1", name=f"d1_{g}_{t}_{c}")
                        nc.vector.tensor_tensor(
                            d1, in0=sigs[c][HID:, :], in1=d0f[HID:, :], op=ALU.mult)
                        nc.vector.tensor_tensor(
                            h_next[:HID, cc], in0=ht, in1=d1, op=ALU.subtract)

                    # emit hidden state (host applies relu + linear head)
                    nc.sync.dma_start(out_d[g, t], h_next[:HID, :])

                # ---------------- schedule ----------------
                # stage 1 for group 0 (DMA-paced; PE interleaves as data lands)
                agg_begin(0)
                for k in range(KCH):
                    agg_k(0, k)
                agg_finish(0)

                # GRU(0) with stage-1(1) sprinkled in; then GRU(1)
                nk = [0]
                agg_begin(1)

                def mk_sprinkle(g, n):
                    fs = []
                    for _ in range(n):
                        if nk[0] < KCH:
                            k = nk[0]
                            nk[0] += 1
                            fs.append(lambda k=k: agg_k(g, k))
                    return fs

                # Staggered-group schedule: while group g's GRU runs, group
                # g+1's aggregation matmuls fill PE slack, and successive
                # groups' recurrences interleave so their independent
                # dependency chains hide each other's latency.
                for t in range(6):
                    gru_step(0, t, mk_sprinkle(1, 14))
                while nk[0] < KCH:
                    agg_k(1, nk[0])
                    nk[0] += 1
                agg_finish(1)
                agg_begin(2)
                nk[0] = 0
                for t in range(6):
                    gru_step(0, 6 + t, mk_sprinkle(2, 7))
                    gru_step(1, t, mk_sprinkle(2, 7))
                while nk[0] < KCH:
                    agg_k(2, nk[0])
                    nk[0] += 1
                agg_finish(2)
                for t in range(6):
                    gru_step(1, 6 + t, [])
                    gru_step(2, t, [])
                for t in range(6):
                    gru_step(2, 6 + t, [])

    nc.compile()
    return nc


def _prep_host(x, edge_index, edge_weight, Wz, bz, Wr, br, Wh, bh,
               Lz_w, Lz_b, Lr_w, Lr_b, Lh_w, Lh_b, lin_w, lin_b):
    """Host-side preprocessing: norm, dense A, feature reorder, weight folding."""
    bf16 = ml_dtypes.bfloat16
    f32 = np.float32

    src = np.asarray(edge_index[0], dtype=np.int64)
    dst = np.asarray(edge_index[1], dtype=np.int64)
    w = np.asarray(edge_weight, dtype=f32)
    loop = np.arange(N_NODES, dtype=np.int64)
    src_a = np.concatenate([src, loop])
    dst_a = np.concatenate([dst, loop])
    w_a = np.concatenate([w, np.ones(N_NODES, f32)])
    deg = np.zeros(N_NODES, f32)
    np.add.at(deg, dst_a, w_a)
    dinv = np.where(deg > 0, 1.0 / np.sqrt(deg), 0.0).astype(f32)
    norm = dinv[src_a] * w_a * dinv[dst_a]

    A = np.zeros((N_SRC_PAD, N_NODES), f32)   # A[src, dst]
    np.add.at(A, (src_a, dst_a), norm)
    A_bf = A.astype(bf16)

    # X rows: [n_src_pad, 96], f = b*24 + t*2 + fin
    X = np.zeros((N_SRC_PAD, B * T * FIN), f32)
    X[:N_NODES] = np.transpose(np.asarray(x, f32), (1, 0, 3, 2)).reshape(N_NODES, -1)
    x_feat = np.ascontiguousarray(
        X.astype(bf16).reshape(KCH, P, B * T * FIN).transpose(1, 0, 2))

    Wz, Wr, Wh = np.asarray(Wz, f32), np.asarray(Wr, f32), np.asarray(Wh, f32)
    Lz_w, Lr_w, Lh_w = np.asarray(Lz_w, f32), np.asarray(Lr_w, f32), np.asarray(Lh_w, f32)
    Az, Ar, Ah = Wz @ Lz_w[:HID], Wr @ Lr_w[:HID], Wh @ Lh_w[:HID]
    Lz2, Lr2, Lh2 = Lz_w[HID:], Lr_w[HID:], Lh_w[HID:]
    bz_f = np.asarray(bz, f32) @ Lz_w[:HID] + np.asarray(Lz_b, f32)
    br_f = np.asarray(br, f32) @ Lr_w[:HID] + np.asarray(Lr_b, f32)
    bh_f = np.asarray(bh, f32) @ Lh_w[:HID] + np.asarray(Lh_b, f32)

    w_zr_np = np.concatenate([np.concatenate([Lr2, Lz2], axis=1),
                              np.concatenate([Ar, Az], axis=1)], axis=0)
    w_h_np = np.concatenate([Lh2, Ah], axis=0)
    common = {
        "x_feat": x_feat,
        "w_zr": np.ascontiguousarray(w_zr_np).astype(bf16),
        "w_h": np.ascontiguousarray(w_h_np).astype(bf16),
        "b_zr": np.concatenate([br_f, bz_f]).astype(f32).reshape(-1, 1),
        "b_h2": np.concatenate([bh_f, bh_f]).astype(f32).reshape(-1, 1),
    }
    in_maps = []
    pad = G * NPG - NPC
    for c in range(N_CORES):
        blk = A_bf[:, c * NPC : (c + 1) * NPC]            # [10112, 1250]
        blk = np.concatenate(
            [blk, np.zeros((N_SRC_PAD, pad), blk.dtype)], axis=1)
        # [P, G, KCH, NPG]
        a_core = np.ascontiguousarray(
            blk.reshape(KCH, P, G, NPG).transpose(1, 2, 0, 3))
        in_maps.append(dict(common, a_t=a_core))
    return in_maps


_CACHED_NC = None


def kernel(**inputs) -> np.ndarray:
    global _CACHED_NC
    from concourse.bass_utils import run_bass_kernel_spmd

    in_maps = _prep_host(**inputs)
    if _CACHED_NC is None:
        _CACHED_NC = build_program()
    res = run_bass_kernel_spmd(_CACHED_NC, in_maps, core_ids=list(range(N_CORES)))

    lin_w = np.asarray(inputs["lin_w"], np.float32)
    lin_b = np.asarray(inputs["lin_b"], np.float32)
    full = np.empty((B, T, N_NODES, OUT), np.float32)
    for c, r in enumerate(res.results):
        hn = np.maximum(r["out"].astype(np.float32), 0.0)   # [G, T, 64, CELLS]
        hn = hn.reshape(G, T, HID, B, NPG).transpose(3, 1, 0, 4, 2)  # b,t,g,n,h
        o = (hn.reshape(-1, HID) @ lin_w + lin_b).reshape(B, T, G * NPG, OUT)
        full[:, :, c * NPC : (c + 1) * NPC, :] = o[:, :, :NPC]
    return full


# revision 11
# speedup vs baseline: 1.4407x; 1.1115x over previous
"""Batched TGCN (GCN-GRU) Trainium2 kernel.

Strategy:
  - GCNConv is linear in x: segment_sum(norm * (X W)[src] -> dst) == (A_norm @ X) W.
    So the graph aggregation A_norm @ X is done ONCE over all B*Fin*T = 96 feature
    columns, shared by all 3 gates and all 12 timesteps.
  - Host: builds the dense normalized adjacency (incl. self loops) in bf16, folds
    the GCN weight into the GRU input weights, reorders x.
  - Device (8 cores, node-sharded 1250 dst nodes/core, zero cross-core comms):
      Stage 1: aggT[96, 1250] = sum_k X_chunk[k].T @ A_T_chunk[k]  (bf16 matmuls,
               fp32 PSUM accumulation), then de-interleave batch into columns.
      Stage 2: 12-step GRU recurrence in feature-major layout [64, 5000]
               (columns = local_node*4 + batch), all state SBUF-resident.
  - Output: [12, 16, 5000] fp32 per core, host reassembles [B, T, N, OUT].
"""

import numpy as np
import ml_dtypes

import concourse.mybir as mybir
import concourse.tile as tile
from concourse import bacc
from concourse.bass import ds

F32 = mybir.dt.float32
BF16 = mybir.dt.bfloat16
AF = mybir.ActivationFunctionType

# Problem constants (hardcoded per contract)
N_NODES = 10000
FIN = 2
HID = 64
OUT = 16
B = 4
T = 12
N_CORES = 8

P = 128
NPC = N_NODES // N_CORES          # 1250 dst nodes per core
NFEAT = B * FIN * T               # 96 aggregation features
N_SRC_PAD = ((N_NODES + P - 1) // P) * P   # 10112
KCH = N_SRC_PAD // P              # 79 k-chunks
KB = 8                            # k-chunks per A-stream DMA
DST_TILE = 512                    # aggregation psum tile width
CHUNK = 1000                      # recurrence column chunk (5 chunks of 1000)


def build_program(npc=NPC, kch=KCH, chunk=CHUNK):
    """Build the per-core Bass program. All 8 cores run the same program."""
    fcol = npc * B
    n_chunks = (fcol + chunk - 1) // chunk
    dst_tiles = []
    o = 0
    while o < npc:
        w = min(DST_TILE, npc - o)
        dst_tiles.append((o, w))
        o += w

    nc = bacc.Bacc("TRN2", target_bir_lowering=False, debug=False)

    def mm_tiled(out, lhsT, rhs, start, stop):
        # matmul free dim must fit one PSUM bank (<=512 fp32 columns)
        w = out.shape[-1]
        for o in range(0, w, 512):
            ww = min(512, w - o)
            nc.tensor.matmul(out[:, o : o + ww], lhsT=lhsT, rhs=rhs[:, o : o + ww],
                             start=start, stop=stop)

    a_t = nc.dram_tensor("a_t", [P, kch, npc], BF16, kind="ExternalInput")
    x_feat = nc.dram_tensor("x_feat", [P, kch, NFEAT], BF16, kind="ExternalInput")
    w_zr = nc.dram_tensor("w_zr", [HID + FIN, 2 * HID], BF16, kind="ExternalInput")
    w_h = nc.dram_tensor("w_h", [HID + FIN, HID], BF16, kind="ExternalInput")
    w_lin = nc.dram_tensor("w_lin", [HID, OUT], BF16, kind="ExternalInput")
    b_zr = nc.dram_tensor("b_zr", [2 * HID, 1], F32, kind="ExternalInput")
    b_h = nc.dram_tensor("b_h", [HID, 1], F32, kind="ExternalInput")
    b_lin = nc.dram_tensor("b_lin", [OUT, 1], F32, kind="ExternalInput")
    out_d = nc.dram_tensor("out", [T, OUT, fcol], F32, kind="ExternalOutput")

    with tile.TileContext(nc) as tc:
        with tc.tile_pool(name="persist", bufs=1) as pp:
            # persistent SBUF tensors
            x_sb = pp.tile([P, kch, NFEAT], BF16, tag="x_sb")
            nc.sync.dma_start(x_sb[:], x_feat[:])

            wzr = pp.tile([HID + FIN, 2 * HID], BF16, tag="wzr")
            nc.sync.dma_start(wzr[:], w_zr[:])
            wh = pp.tile([HID + FIN, HID], BF16, tag="wh")
            nc.sync.dma_start(wh[:], w_h[:])
            wlin = pp.tile([HID, OUT], BF16, tag="wlin")
            nc.sync.dma_start(wlin[:], w_lin[:])
            brz_t = pp.tile([2 * HID, 1], F32, tag="brz_t")
            nc.sync.dma_start(brz_t[:], b_zr[:])
            bh = pp.tile([HID, 1], F32, tag="bh")
            nc.sync.dma_start(bh[:], b_h[:])
            blin = pp.tile([OUT, 1], F32, tag="blin")
            nc.sync.dma_start(blin[:], b_lin[:])

            agg_nodes = pp.tile([NFEAT, npc], BF16, tag="agg_nodes")
            aggT = pp.tile([FIN * T, fcol], BF16, tag="aggT")
            h_bufs = [pp.tile([HID + FIN, fcol], BF16, tag=f"h{i}", name=f"h{i}") for i in range(2)]
            rh_bufs = [pp.tile([HID + FIN, fcol], BF16, tag=f"rh{i}", name=f"rh{i}") for i in range(2)]
            nc.gpsimd.memset(h_bufs[0][:HID, :], 0.0)
            out_sb = [pp.tile([OUT, fcol], F32, tag=f"osb{i}", name=f"osb{i}") for i in range(2)]

            # ---------------- Stage 1: aggregation ----------------
            with (
                tc.tile_pool(name="astream", bufs=3) as ap_,
                tc.tile_pool(name="apsum", bufs=1, space="PSUM") as aps,
            ):
                psums = [aps.tile([NFEAT, w], F32, tag=f"agp{i}", name=f"agp{i}")
                         for i, (_, w) in enumerate(dst_tiles)]
                n_ktiles = (kch + KB - 1) // KB
                for kt in range(n_ktiles):
                    k0 = kt * KB
                    kb = min(KB, kch - k0)
                    a_sb = ap_.tile([P, KB, npc], BF16, tag="a_sb")
                    nc.sync.dma_start(a_sb[:, :kb], a_t[:, k0 : k0 + kb, :])
                    for kl in range(kb):
                        k = k0 + kl
                        for i, (doff, w) in enumerate(dst_tiles):
                            nc.tensor.matmul(
                                psums[i][:],
                                lhsT=x_sb[:, k, :],
                                rhs=a_sb[:, kl, ds(doff, w)],
                                start=(k == 0),
                                stop=(k == kch - 1),
                            )
                for i, (doff, w) in enumerate(dst_tiles):
                    nc.vector.tensor_copy(agg_nodes[:, ds(doff, w)], psums[i][:])

            # de-interleave batch, b-major columns (col = b*npc + n), all contiguous
            # aggT[t*2+fin, b*npc + n] = agg_nodes[b*24 + t*2 + fin, n]
            for b in range(B):
                nc.sync.dma_start(
                    aggT[:, ds(b * npc, npc)],
                    agg_nodes[b * (FIN * T) : (b + 1) * (FIN * T), :],
                )

            # ---------------- Stage 2: GRU recurrence ----------------
            # 4-stage software-pipelined emission: engines are in-order FIFOs,
            # so stages of successive (t, chunk) iterations are interleaved to
            # keep every engine queue filled with ready work.
            with (
                tc.tile_pool(name="work", bufs=8) as wp,
                tc.tile_pool(name="pzr", bufs=2, space="PSUM") as pzr_pool,
                tc.tile_pool(name="phl", bufs=2, space="PSUM") as phl_pool,
            ):
                IT = [(t, c) for t in range(T) for c in range(n_chunks)]
                sigs, hts = {}, {}

                def S1(it):
                    t, c = it
                    h_prev = h_bufs[t % 2]
                    if c == 0:
                        nc.sync.dma_start(h_prev[HID:, :], aggT[2 * t : 2 * t + 2, :])
                        nc.sync.dma_start(rh_bufs[t % 2][HID:, :], aggT[2 * t : 2 * t + 2, :])
                    cw = min(chunk, fcol - c * chunk)
                    cc = ds(c * chunk, cw)
                    pz = pzr_pool.tile([2 * HID, chunk], F32, tag="pz", name="pz")[:, :cw]
                    mm_tiled(pz, wzr[:], h_prev[:, cc], True, True)
                    sig = wp.tile([2 * HID, chunk], BF16, tag="sig", name="sig")[:, :cw]
                    nc.scalar.activation(sig, pz, AF.Sigmoid, bias=brz_t[:, 0:1])
                    sigs[it] = sig

                def S2(it):
                    t, c = it
                    h_prev, rh_full = h_bufs[t % 2], rh_bufs[t % 2]
                    cw = min(chunk, fcol - c * chunk)
                    cc = ds(c * chunk, cw)
                    sig = sigs[it]
                    nc.vector.tensor_tensor(
                        rh_full[:HID, cc], in0=sig[:HID, :], in1=h_prev[:HID, cc],
                        op=mybir.AluOpType.mult)
                    ph = phl_pool.tile([HID, chunk], F32, tag="phl", name="ph")[:, :cw]
                    mm_tiled(ph, wh[:], rh_full[:, cc], True, True)
                    ht = wp.tile([HID, chunk], BF16, tag="ht", name="ht")[:, :cw]
                    nc.scalar.activation(ht, ph, AF.Tanh, bias=bh[:, 0:1])
                    hts[it] = ht

                def S3(it):
                    t, c = it
                    h_prev, h_next = h_bufs[t % 2], h_bufs[(t + 1) % 2]
                    cw = min(chunk, fcol - c * chunk)
                    cc = ds(c * chunk, cw)
                    sig, ht = sigs.pop(it), hts.pop(it)
                    d0f = wp.tile([2 * HID, chunk], BF16, tag="d0f", name="d0f")[:, :cw]
                    nc.vector.tensor_tensor(d0f[HID:, :], in0=ht, in1=h_prev[:HID, cc],
                                            op=mybir.AluOpType.subtract)
                    d1 = wp.tile([HID, chunk], BF16, tag="d1", name="d1")[:, :cw]
                    nc.vector.tensor_tensor(d1, in0=sig[HID:, :], in1=d0f[HID:, :],
                                            op=mybir.AluOpType.mult)
                    nc.vector.tensor_tensor(h_next[:HID, cc], in0=ht, in1=d1,
                                            op=mybir.AluOpType.subtract)

                def S4(it):
                    t, c = it
                    h_next, ot_sb = h_bufs[(t + 1) % 2], out_sb[t % 2]
                    cw = min(chunk, fcol - c * chunk)
                    cc = ds(c * chunk, cw)
                    rl = wp.tile([HID, chunk], BF16, tag="rl", name="rl")[:, :cw]
                    nc.vector.tensor_scalar_max(rl, h_next[:HID, cc], 0.0)
                    plin = phl_pool.tile([HID, chunk], F32, tag="phl", name="plin")[:OUT, :cw]
                    mm_tiled(plin, wlin[:], rl, True, True)
                    nc.scalar.activation(ot_sb[:, cc], plin, AF.Identity,
                                         bias=blin[:, 0:1])
                    if c == n_chunks - 1:
                        nc.sync.dma_start(out_d[t], ot_sb[:])

                n = len(IT)
                for i in range(n + 3):
                    if i < n:
                        S1(IT[i])
                    if 1 <= i < n + 1:
                        S2(IT[i - 1])
                    if 2 <= i < n + 2:
                        S3(IT[i - 2])
                    if 3 <= i < n + 3:
                        S4(IT[i - 3])

    nc.compile()
    return nc


def _prep_host(x, edge_index, edge_weight, Wz, bz, Wr, br, Wh, bh,
               Lz_w, Lz_b, Lr_w, Lr_b, Lh_w, Lh_b, lin_w, lin_b,
               n_nodes=N_NODES, npc=NPC, n_cores=N_CORES):
    """Host-side preprocessing: norm, dense A, feature reorder, weight folding."""
    bf16 = ml_dtypes.bfloat16
    n_src_pad = ((n_nodes + P - 1) // P) * P
    kch = n_src_pad // P

    src = np.asarray(edge_index[0], dtype=np.int64)
    dst = np.asarray(edge_index[1], dtype=np.int64)
    w = np.asarray(edge_weight, dtype=np.float32)
    loop = np.arange(n_nodes, dtype=np.int64)
    src_a = np.concatenate([src, loop])
    dst_a = np.concatenate([dst, loop])
    w_a = np.concatenate([w, np.ones(n_nodes, np.float32)])
    deg = np.zeros(n_nodes, np.float32)
    np.add.at(deg, dst_a, w_a)
    dinv = np.where(deg > 0, 1.0 / np.sqrt(deg), 0.0).astype(np.float32)
    norm = dinv[src_a] * w_a * dinv[dst_a]

    A = np.zeros((n_src_pad, n_nodes), np.float32)   # A[src, dst]
    np.add.at(A, (src_a, dst_a), norm)
    A_bf = A.astype(bf16)

    # X rows: [n_src_pad, 96], f = b*24 + t*2 + fin  (b outer, fin inner)
    X = np.zeros((n_src_pad, B * T * FIN), np.float32)
    X[:n_nodes] = np.transpose(np.asarray(x, np.float32), (1, 0, 3, 2)).reshape(n_nodes, -1)
    x_feat = np.ascontiguousarray(
        X.astype(bf16).reshape(kch, P, B * T * FIN).transpose(1, 0, 2))

    f32 = np.float32
    Wz, Wr, Wh = np.asarray(Wz, f32), np.asarray(Wr, f32), np.asarray(Wh, f32)
    Lz_w, Lr_w, Lh_w = np.asarray(Lz_w, f32), np.asarray(Lr_w, f32), np.asarray(Lh_w, f32)
    Az, Ar, Ah = Wz @ Lz_w[:HID], Wr @ Lr_w[:HID], Wh @ Lh_w[:HID]
    Lz2, Lr2, Lh2 = Lz_w[HID:], Lr_w[HID:], Lh_w[HID:]
    bz_f = np.asarray(bz, f32) @ Lz_w[:HID] + np.asarray(Lz_b, f32)
    br_f = np.asarray(br, f32) @ Lr_w[:HID] + np.asarray(Lr_b, f32)
    bh_f = np.asarray(bh, f32) @ Lh_w[:HID] + np.asarray(Lh_b, f32)

    w_zr_np = np.concatenate([np.concatenate([Lr2, Lz2], axis=1),
                              np.concatenate([Ar, Az], axis=1)], axis=0)
    w_h_np = np.concatenate([Lh2, Ah], axis=0)
    common = {
        "x_feat": x_feat,
        "w_zr": np.ascontiguousarray(w_zr_np).astype(bf16),
        "w_h": np.ascontiguousarray(w_h_np).astype(bf16),
        "w_lin": np.asarray(lin_w, f32).astype(bf16),
        "b_zr": np.concatenate([br_f, bz_f]).astype(f32).reshape(-1, 1),
        "b_h": bh_f.astype(f32).reshape(-1, 1),
        "b_lin": np.asarray(lin_b, f32).reshape(-1, 1),
    }
    in_maps = []
    for c in range(n_cores):
        a_core = np.ascontiguousarray(
            A_bf[:, c * npc : (c + 1) * npc].reshape(kch, P, npc).transpose(1, 0, 2))
        in_maps.append(dict(common, a_t=a_core))
    return in_maps


_CACHED_NC = None


def kernel(**inputs) -> np.ndarray:
    global _CACHED_NC
    from concourse.bass_utils import run_bass_kernel_spmd

    in_maps = _prep_host(**inputs)
    if _CACHED_NC is None:
        _CACHED_NC = build_program()
    res = run_bass_kernel_spmd(_CACHED_NC, in_maps, core_ids=list(range(N_CORES)))

    full = np.empty((B, T, N_NODES, OUT), np.float32)
    for c, r in enumerate(res.results):
        o = r["out"].reshape(T, OUT, B, NPC)          # [t, o, b, n_local]
        full[:, :, c * NPC : (c + 1) * NPC, :] = o.transpose(2, 0, 3, 1)
    return full



# revision 12
# speedup vs baseline: 1.5470x; 1.0738x over previous
"""Batched TGCN (GCN-GRU) Trainium2 kernel.

Strategy:
  - GCNConv is linear in x: segment_sum(norm * (X W)[src] -> dst) == (A_norm @ X) W.
    So the graph aggregation A_norm @ X is done ONCE over all B*Fin*T = 96 feature
    columns, shared by all 3 gates and all 12 timesteps.
  - Host: builds the dense normalized adjacency (incl. self loops) in bf16, folds
    the GCN weight into the GRU input weights, reorders x.
  - Device (8 cores, node-sharded 1250 dst nodes/core, zero cross-core comms):
      Stage 1: aggT[96, 1250] = sum_k X_chunk[k].T @ A_T_chunk[k]  (bf16 matmuls,
               fp32 PSUM accumulation), then de-interleave batch into columns.
      Stage 2: 12-step GRU recurrence in feature-major layout [64, 5000]
               (columns = local_node*4 + batch), all state SBUF-resident.
  - Output: [12, 16, 5000] fp32 per core, host reassembles [B, T, N, OUT].
"""

import numpy as np
import ml_dtypes

import concourse.mybir as mybir
import concourse.tile as tile
from concourse import bacc
from concourse.bass import ds

F32 = mybir.dt.float32
BF16 = mybir.dt.bfloat16
AF = mybir.ActivationFunctionType

# Problem constants (hardcoded per contract)
N_NODES = 10000
FIN = 2
HID = 64
OUT = 16
B = 4
T = 12
N_CORES = 8

P = 128
NPC = N_NODES // N_CORES          # 1250 dst nodes per core
NFEAT = B * FIN * T               # 96 aggregation features
N_SRC_PAD = ((N_NODES + P - 1) // P) * P   # 10112
KCH = N_SRC_PAD // P              # 79 k-chunks
KB = 8                            # k-chunks per A-stream DMA
DST_TILE = 512                    # aggregation psum tile width
CHUNK = 1000                      # recurrence column chunk (5 chunks of 1000)


def build_program(npc=NPC, kch=KCH, chunk=CHUNK):
    """Build the per-core Bass program. All 8 cores run the same program."""
    fcol = npc * B
    n_chunks = (fcol + chunk - 1) // chunk
    dst_tiles = []
    o = 0
    while o < npc:
        w = min(DST_TILE, npc - o)
        dst_tiles.append((o, w))
        o += w

    nc = bacc.Bacc("TRN2", target_bir_lowering=False, debug=False)

    def mm_tiled(out, lhsT, rhs, start, stop):
        # matmul free dim must fit one PSUM bank (<=512 fp32 columns)
        w = out.shape[-1]
        for o in range(0, w, 512):
            ww = min(512, w - o)
            nc.tensor.matmul(out[:, o : o + ww], lhsT=lhsT, rhs=rhs[:, o : o + ww],
                             start=start, stop=stop)

    a_t = nc.dram_tensor("a_t", [P, kch, npc], BF16, kind="ExternalInput")
    x_feat = nc.dram_tensor("x_feat", [P, kch, NFEAT], BF16, kind="ExternalInput")
    w_zr = nc.dram_tensor("w_zr", [HID + FIN, 2 * HID], BF16, kind="ExternalInput")
    w_h = nc.dram_tensor("w_h", [HID + FIN, HID], BF16, kind="ExternalInput")
    b_zr = nc.dram_tensor("b_zr", [2 * HID, 1], F32, kind="ExternalInput")
    b_h = nc.dram_tensor("b_h", [HID, 1], F32, kind="ExternalInput")
    out_d = nc.dram_tensor("out", [T, HID, fcol], BF16, kind="ExternalOutput")

    with tile.TileContext(nc) as tc:
        with tc.tile_pool(name="persist", bufs=1) as pp:
            # persistent SBUF tensors
            x_sb = pp.tile([P, kch, NFEAT], BF16, tag="x_sb")
            nc.sync.dma_start(x_sb[:], x_feat[:])

            wzr = pp.tile([HID + FIN, 2 * HID], BF16, tag="wzr")
            nc.sync.dma_start(wzr[:], w_zr[:])
            wh = pp.tile([HID + FIN, HID], BF16, tag="wh")
            nc.sync.dma_start(wh[:], w_h[:])
            brz_t = pp.tile([2 * HID, 1], F32, tag="brz_t")
            nc.sync.dma_start(brz_t[:], b_zr[:])
            bh = pp.tile([HID, 1], F32, tag="bh")
            nc.sync.dma_start(bh[:], b_h[:])

            agg_nodes = pp.tile([NFEAT, npc], BF16, tag="agg_nodes")
            aggT = pp.tile([FIN * T, fcol], BF16, tag="aggT")
            h_bufs = [pp.tile([HID + FIN, fcol], BF16, tag=f"h{i}", name=f"h{i}") for i in range(2)]
            rh_bufs = [pp.tile([HID + FIN, fcol], BF16, tag=f"rh{i}", name=f"rh{i}") for i in range(2)]
            nc.gpsimd.memset(h_bufs[0][:HID, :], 0.0)

            # ---------------- Stage 1: aggregation ----------------
            with (
                tc.tile_pool(name="astream", bufs=3) as ap_,
                tc.tile_pool(name="apsum", bufs=1, space="PSUM") as aps,
            ):
                psums = [aps.tile([NFEAT, w], F32, tag=f"agp{i}", name=f"agp{i}")
                         for i, (_, w) in enumerate(dst_tiles)]
                n_ktiles = (kch + KB - 1) // KB
                for kt in range(n_ktiles):
                    k0 = kt * KB
                    kb = min(KB, kch - k0)
                    a_sb = ap_.tile([P, KB, npc], BF16, tag="a_sb")
                    nc.sync.dma_start(a_sb[:, :kb], a_t[:, k0 : k0 + kb, :])
                    for kl in range(kb):
                        k = k0 + kl
                        for i, (doff, w) in enumerate(dst_tiles):
                            nc.tensor.matmul(
                                psums[i][:],
                                lhsT=x_sb[:, k, :],
                                rhs=a_sb[:, kl, ds(doff, w)],
                                start=(k == 0),
                                stop=(k == kch - 1),
                            )
                for i, (doff, w) in enumerate(dst_tiles):
                    nc.vector.tensor_copy(agg_nodes[:, ds(doff, w)], psums[i][:])

            # de-interleave batch, b-major columns (col = b*npc + n), all contiguous
            # aggT[t*2+fin, b*npc + n] = agg_nodes[b*24 + t*2 + fin, n]
            for b in range(B):
                nc.sync.dma_start(
                    aggT[:, ds(b * npc, npc)],
                    agg_nodes[b * (FIN * T) : (b + 1) * (FIN * T), :],
                )

            # ---------------- Stage 2: GRU recurrence ----------------
            # 4-stage software-pipelined emission: engines are in-order FIFOs,
            # so stages of successive (t, chunk) iterations are interleaved to
            # keep every engine queue filled with ready work.
            with (
                tc.tile_pool(name="work", bufs=8) as wp,
                tc.tile_pool(name="pzr", bufs=2, space="PSUM") as pzr_pool,
                tc.tile_pool(name="phl", bufs=2, space="PSUM") as phl_pool,
            ):
                IT = [(t, c) for t in range(T) for c in range(n_chunks)]
                sigs, hts = {}, {}

                def S1(it):
                    t, c = it
                    h_prev = h_bufs[t % 2]
                    if c == 0:
                        nc.sync.dma_start(h_prev[HID:, :], aggT[2 * t : 2 * t + 2, :])
                        nc.sync.dma_start(rh_bufs[t % 2][HID:, :], aggT[2 * t : 2 * t + 2, :])
                    cw = min(chunk, fcol - c * chunk)
                    cc = ds(c * chunk, cw)
                    pz = pzr_pool.tile([2 * HID, chunk], F32, tag="pz", name="pz")[:, :cw]
                    mm_tiled(pz, wzr[:], h_prev[:, cc], True, True)
                    sig = wp.tile([2 * HID, chunk], BF16, tag="sig", name="sig")[:, :cw]
                    nc.scalar.activation(sig, pz, AF.Sigmoid, bias=brz_t[:, 0:1])
                    sigs[it] = sig

                def S2(it):
                    t, c = it
                    h_prev, rh_full = h_bufs[t % 2], rh_bufs[t % 2]
                    cw = min(chunk, fcol - c * chunk)
                    cc = ds(c * chunk, cw)
                    sig = sigs[it]
                    nc.vector.tensor_tensor(
                        rh_full[:HID, cc], in0=sig[:HID, :], in1=h_prev[:HID, cc],
                        op=mybir.AluOpType.mult)
                    ph = phl_pool.tile([HID, chunk], F32, tag="phl", name="ph")[:, :cw]
                    mm_tiled(ph, wh[:], rh_full[:, cc], True, True)
                    ht = wp.tile([HID, chunk], BF16, tag="ht", name="ht")[:, :cw]
                    nc.scalar.activation(ht, ph, AF.Tanh, bias=bh[:, 0:1])
                    hts[it] = ht

                def S3(it):
                    t, c = it
                    h_prev, h_next = h_bufs[t % 2], h_bufs[(t + 1) % 2]
                    cw = min(chunk, fcol - c * chunk)
                    cc = ds(c * chunk, cw)
                    sig, ht = sigs.pop(it), hts.pop(it)
                    d0f = wp.tile([2 * HID, chunk], BF16, tag="d0f", name="d0f")[:, :cw]
                    nc.vector.tensor_tensor(d0f[HID:, :], in0=ht, in1=h_prev[:HID, cc],
                                            op=mybir.AluOpType.subtract)
                    d1 = wp.tile([HID, chunk], BF16, tag="d1", name="d1")[:, :cw]
                    nc.vector.tensor_tensor(d1, in0=sig[HID:, :], in1=d0f[HID:, :],
                                            op=mybir.AluOpType.mult)
                    nc.vector.tensor_tensor(h_next[:HID, cc], in0=ht, in1=d1,
                                            op=mybir.AluOpType.subtract)

                def S4(it):
                    t, c = it
                    if c == n_chunks - 1:
                        nc.sync.dma_start(out_d[t], h_bufs[(t + 1) % 2][:HID, :])

                n = len(IT)
                for i in range(n + 3):
                    if i < n:
                        S1(IT[i])
                    if 1 <= i < n + 1:
                        S2(IT[i - 1])
                    if 2 <= i < n + 2:
                        S3(IT[i - 2])
                    if 3 <= i < n + 3:
                        S4(IT[i - 3])

    nc.compile()
    return nc


def _prep_host(x, edge_index, edge_weight, Wz, bz, Wr, br, Wh, bh,
               Lz_w, Lz_b, Lr_w, Lr_b, Lh_w, Lh_b, lin_w, lin_b,
               n_nodes=N_NODES, npc=NPC, n_cores=N_CORES):
    """Host-side preprocessing: norm, dense A, feature reorder, weight folding."""
    bf16 = ml_dtypes.bfloat16
    n_src_pad = ((n_nodes + P - 1) // P) * P
    kch = n_src_pad // P

    src = np.asarray(edge_index[0], dtype=np.int64)
    dst = np.asarray(edge_index[1], dtype=np.int64)
    w = np.asarray(edge_weight, dtype=np.float32)
    loop = np.arange(n_nodes, dtype=np.int64)
    src_a = np.concatenate([src, loop])
    dst_a = np.concatenate([dst, loop])
    w_a = np.concatenate([w, np.ones(n_nodes, np.float32)])
    deg = np.zeros(n_nodes, np.float32)
    np.add.at(deg, dst_a, w_a)
    dinv = np.where(deg > 0, 1.0 / np.sqrt(deg), 0.0).astype(np.float32)
    norm = dinv[src_a] * w_a * dinv[dst_a]

    A = np.zeros((n_src_pad, n_nodes), np.float32)   # A[src, dst]
    np.add.at(A, (src_a, dst_a), norm)
    A_bf = A.astype(bf16)

    # X rows: [n_src_pad, 96], f = b*24 + t*2 + fin  (b outer, fin inner)
    X = np.zeros((n_src_pad, B * T * FIN), np.float32)
    X[:n_nodes] = np.transpose(np.asarray(x, np.float32), (1, 0, 3, 2)).reshape(n_nodes, -1)
    x_feat = np.ascontiguousarray(
        X.astype(bf16).reshape(kch, P, B * T * FIN).transpose(1, 0, 2))

    f32 = np.float32
    Wz, Wr, Wh = np.asarray(Wz, f32), np.asarray(Wr, f32), np.asarray(Wh, f32)
    Lz_w, Lr_w, Lh_w = np.asarray(Lz_w, f32), np.asarray(Lr_w, f32), np.asarray(Lh_w, f32)
    Az, Ar, Ah = Wz @ Lz_w[:HID], Wr @ Lr_w[:HID], Wh @ Lh_w[:HID]
    Lz2, Lr2, Lh2 = Lz_w[HID:], Lr_w[HID:], Lh_w[HID:]
    bz_f = np.asarray(bz, f32) @ Lz_w[:HID] + np.asarray(Lz_b, f32)
    br_f = np.asarray(br, f32) @ Lr_w[:HID] + np.asarray(Lr_b, f32)
    bh_f = np.asarray(bh, f32) @ Lh_w[:HID] + np.asarray(Lh_b, f32)

    w_zr_np = np.concatenate([np.concatenate([Lr2, Lz2], axis=1),
                              np.concatenate([Ar, Az], axis=1)], axis=0)
    w_h_np = np.concatenate([Lh2, Ah], axis=0)
    common = {
        "x_feat": x_feat,
        "w_zr": np.ascontiguousarray(w_zr_np).astype(bf16),
        "w_h": np.ascontiguousarray(w_h_np).astype(bf16),
        "b_zr": np.concatenate([br_f, bz_f]).astype(f32).reshape(-1, 1),
        "b_h": bh_f.astype(f32).reshape(-1, 1),
    }
    in_maps = []
    for c in range(n_cores):
        a_core = np.ascontiguousarray(
            A_bf[:, c * npc : (c + 1) * npc].reshape(kch, P, npc).transpose(1, 0, 2))
        in_maps.append(dict(common, a_t=a_core))
    return in_maps


_CACHED_NC = None


def kernel(**inputs) -> np.ndarray:
    global _CACHED_NC
    from concourse.bass_utils import run_bass_kernel_spmd

    in_maps = _prep_host(**inputs)
    if _CACHED_NC is None:
        _CACHED_NC = build_program()
    res = run_bass_kernel_spmd(_CACHED_NC, in_maps, core_ids=list(range(N_CORES)))

    lin_w = np.asarray(inputs["lin_w"], np.float32)
    lin_b = np.asarray(inputs["lin_b"], np.float32)
    full = np.empty((B, T, N_NODES, OUT), np.float32)
    for c, r in enumerate(res.results):
        hn = np.maximum(r["out"].astype(np.float32), 0.0)   # [T, 64, fcol]
        hn = hn.reshape(T, HID, B, NPC).transpose(2, 0, 3, 1)   # b,t,n,h
        o = hn.reshape(-1, HID) @ lin_w + lin_b
        full[:, :, c * NPC : (c + 1) * NPC, :] = o.reshape(B, T, NPC, OUT)
    return full

